# revision 1
# baseline (speedup 1.0000x reference)
"""AVFusion kernel for 8 trn2 NeuronCores.

Structure (per core, data-parallel over bs: 2 batches/core):
  All activations kept transposed (d on partitions as [128, d/128, tokens]).
  Prologue: A'=wA@A.T+bA etc; k,v,q projections; 2-way softmax collapses to
  p = sigmoid((q.kA - q.kV)/sqrt(dk)); p flattened to a row vector.
  Main loop over 512-token blocks (b, s-block of 8, all 64 g):
    PB = ones.T @ p_row (partition-broadcast of p via PE)
    x = vV + PB * (vA - vV)        (DVE, per head chunk)
    y1 = relu(w1 @ x + b1)         (PE + ACT)
    out = w2 @ y1 + b2             (PE + DVE)
  Output written transposed; host untransposes.
"""

import numpy as np

BS, NSEG, NSEN, D, H, DK = 16, 64, 32, 1024, 8, 128
NCORES = 8
BPC = BS // NCORES          # batches per core = 2
TOK_AV = BPC * NSEG         # 128
TOK_S = BPC * NSEN          # 64
TOK_OUT = BPC * NSEN * NSEG  # 4096
KC = D // 128               # 8 d-chunks
BLK = 512                   # tokens per main-loop block
SBLK = BLK // NSEG          # 8 sentence rows per block
NBLK = TOK_OUT // BLK       # 8 blocks per core
SCALE = 1.0 / np.sqrt(np.float32(DK))

_CACHE = {}


def _build_nc():
    import concourse.bass as bass
    import concourse.mybir as mybir
    import concourse.tile as tile
    from concourse import bacc
    from contextlib import ExitStack

    fp32 = mybir.dt.float32
    AF = mybir.ActivationFunctionType
    ALU = mybir.AluOpType

    nc = bacc.Bacc(None, target_bir_lowering=False)

    # ---- per-core DRAM I/O (host pre-transposed layouts) ----
    bf16d = mybir.dt.bfloat16
    AT = nc.dram_tensor("AT", [128, KC, TOK_AV], bf16d, kind="ExternalInput")
    VT = nc.dram_tensor("VT", [128, KC, TOK_AV], bf16d, kind="ExternalInput")
    ST = nc.dram_tensor("ST", [128, KC, TOK_S], bf16d, kind="ExternalInput")
    wts = {}
    for w in ["wA", "wV", "wS", "wq", "wk", "wv", "w1", "w2"]:
        wts[w] = nc.dram_tensor(w, [128, KC, D], bf16d, kind="ExternalInput")
    bias = {}
    for b in ["bA", "bV", "bS", "bq", "bk", "bv", "b1", "b2"]:
        bias[b] = nc.dram_tensor(b, [128, KC], fp32, kind="ExternalInput")
    OUT = nc.dram_tensor("OUT", [128, KC, TOK_OUT], fp32, kind="ExternalOutput")

    with tile.TileContext(nc) as tc, ExitStack() as ctx:
        # persistent small tensors
        pers = ctx.enter_context(tc.tile_pool(name="pers", bufs=1))
        # biases in SBUF [128, KC]
        bsb = {}
        for b in bias:
            bsb[b] = pers.tile([128, KC], fp32, tag=f"b_{b}", name=f"b_{b}")
            nc.sync.dma_start(bsb[b][:], bias[b][:])
        bf16 = mybir.dt.bfloat16
        ones = pers.tile([128, 128], bf16, tag="ones", name="ones")
        nc.vector.memset(ones[:], 1.0)

        dVT = pers.tile([128, KC, TOK_AV], bf16, tag="dVT", name="dVT")
        # p values, flattened per (b, h, s-block) combo into a 512-wide row on
        # partition 32*(combo%4), column (combo//4)*512  (matmul rhs base
        # partition must be one of {0,32,64,96} and match lhsT's)
        n_combo = BPC * H * (NSEN // SBLK)
        pw = -(-n_combo // 3) * BLK
        pflat = pers.tile([128, pw], bf16, tag="pflat", name="pflat")

        # main-loop weights: start DMA early, they are consumed last
        wm = ctx.enter_context(tc.tile_pool(name="wm", bufs=1))
        w1sb = wm.tile([128, KC, D], bf16, tag="w1", name="w1")
        w2sb = wm.tile([128, KC, D], bf16, tag="w2", name="w2")

        # v-projection of the AV pair persists into the main loop:
        # cols [0:128] = vA tokens, [128:256] = vV tokens
        vAVT = pers.tile([128, KC, 2 * TOK_AV], bf16, tag="vAVT", name="vAVT")

        # ---------------- prologue ----------------
        with tc.tile_pool(name="wp", bufs=3) as wp, \
             tc.tile_pool(name="acts", bufs=1) as acts, \
             tc.tile_pool(name="ppsum", bufs=4, space="PSUM") as ppsum, \
             tc.tile_pool(name="lpsum", bufs=2, space="PSUM") as lpsum, \
             tc.tile_pool(name="ptmp", bufs=2) as ptmp:

            ATs = acts.tile([128, KC, TOK_AV], bf16, tag="ATs", name="ATs")
            VTs = acts.tile([128, KC, TOK_AV], bf16, tag="VTs", name="VTs")
            STs = acts.tile([128, KC, TOK_S], bf16, tag="STs", name="STs")

            def dma_chunked(dst, src):
                # per-k-chunk DMAs spread across HW queues for parallelism
                for k in range(KC):
                    nc.sync.dma_start(dst[:, k], src[:, k])

            def wtile(wname):
                t = wp.tile([128, KC, D], bf16, tag="w", name="w")
                dma_chunked(t, wts[wname])
                return t

            def proj(dst, wt_sb, src, bias_tile):
                """dst[:, m, :] = w @ src + b   (all transposed layout)."""
                ntok = src.shape[2]
                for m in range(KC):
                    ps = ppsum.tile([128, 512], fp32, tag="proj_ps", name="proj_ps")
                    for k in range(KC):
                        nc.tensor.matmul(
                            ps[:, :ntok],
                            wt_sb[:, k, m * 128:(m + 1) * 128],
                            src[:, k, :],
                            start=(k == 0), stop=(k == KC - 1),
                        )
                    nc.vector.tensor_tensor(
                        dst[:, m, :], ps[:, :ntok],
                        bias_tile[:, m, None].to_broadcast([128, ntok]),
                        ALU.add,
                    )

            # A'/V' side by side so k/v projections run at N=256
            AV2T = acts.tile([128, KC, 2 * TOK_AV], bf16, tag="AV2T", name="AV2T")
            S2T = acts.tile([128, KC, TOK_S], bf16, tag="S2T", name="S2T")
            kAVT = acts.tile([128, KC, 2 * TOK_AV], bf16, tag="kAVT", name="kAVT")
            qT = acts.tile([128, KC, TOK_S], bf16, tag="qT", name="qT")
            qTn = acts.tile([128, KC, TOK_S], bf16, tag="qTn", name="qTn")

            wAs = wtile("wA")
            dma_chunked(ATs, AT)
            dma_chunked(VTs, VT)
            dma_chunked(STs, ST)
            proj(AV2T[:, :, :TOK_AV], wAs, ATs, bsb["bA"])
            wVs = wtile("wV")
            proj(AV2T[:, :, TOK_AV:], wVs, VTs, bsb["bV"])
            wSs = wtile("wS")
            proj(S2T, wSs, STs, bsb["bS"])
            wks = wtile("wk")
            proj(kAVT, wks, AV2T, bsb["bk"])
            wqs = wtile("wq")
            proj(qT, wqs, S2T, bsb["bq"])
            nc.vector.tensor_scalar_mul(qTn[:], qT[:], -1.0)

            wvs = wtile("wv")
            dma_chunked(w1sb, wts["w1"])
            dma_chunked(w2sb, wts["w2"])

            # logits + sigmoid + flatten:  p = sigmoid((q.kA - q.kV)*SCALE)
            # one PSUM bank per batch holds all 8 heads' logits
            for b in range(BPC):
                lg = lpsum.tile([NSEN, H * NSEG], fp32, tag="lg", name="lg")
                for h in range(H):
                    qs = qT[:, h, b * NSEN:(b + 1) * NSEN]
                    qsn = qTn[:, h, b * NSEN:(b + 1) * NSEN]
                    hs = slice(h * NSEG, (h + 1) * NSEG)
                    nc.tensor.matmul(lg[:, hs], qs,
                                     kAVT[:, h, b * NSEG:(b + 1) * NSEG],
                                     start=True, stop=False)
                    nc.tensor.matmul(lg[:, hs], qsn,
                                     kAVT[:, h, TOK_AV + b * NSEG:TOK_AV + (b + 1) * NSEG],
                                     start=False, stop=True)
                psb = ptmp.tile([NSEN, H * NSEG], bf16, tag="psb", name="psb")
                nc.scalar.activation(psb[:], lg[:], AF.Sigmoid, scale=float(SCALE))
                for h in range(H):
                    for sb in range(NSEN // SBLK):
                        combo = (b * H + h) * (NSEN // SBLK) + sb
                        q0 = 32 * (combo % 3)
                        off = (combo // 3) * BLK
                        nc.sync.dma_start(
                            pflat[q0:q0 + 1, off:off + BLK],
                            psb[sb * SBLK:(sb + 1) * SBLK, h * NSEG:(h + 1) * NSEG])

            # v projection (N=256) + dV = vA - vV, overlapping logits epilogue
            proj(vAVT, wvs, AV2T, bsb["bv"])
            for m in range(KC):
                nc.vector.tensor_tensor(dVT[:, m, :], vAVT[:, m, :TOK_AV],
                                        vAVT[:, m, TOK_AV:], ALU.subtract)

        # ---------------- main loop ----------------
        with tc.tile_pool(name="blk", bufs=2) as blkp, \
             tc.tile_pool(name="pbps", bufs=3, space="PSUM") as pbps, \
             tc.tile_pool(name="f1ps", bufs=2, space="PSUM") as f1ps, \
             tc.tile_pool(name="f2ps", bufs=2, space="PSUM") as f2ps:

            pending = None  # (xblk, tok0) awaiting FF

            def emit_ff(xblk, tok0):
                y1 = blkp.tile([128, KC, BLK], bf16, tag="y1blk", name="y1blk")
                for m in range(KC):
                    ps = f1ps.tile([128, BLK], fp32, tag="f1", name="f1")
                    for k in range(KC):
                        nc.tensor.matmul(ps[:], w1sb[:, k, m * 128:(m + 1) * 128],
                                         xblk[:, k, :],
                                         start=(k == 0), stop=(k == KC - 1))
                    nc.scalar.activation(y1[:, m, :], ps[:], AF.Relu,
                                         bias=bsb["b1"][:, m:m + 1])
                ob = blkp.tile([128, KC, BLK], fp32, tag="oblk", name="oblk")
                for m in range(KC):
                    ps = f2ps.tile([128, BLK], fp32, tag="f2", name="f2")
                    for k in range(KC):
                        nc.tensor.matmul(ps[:], w2sb[:, k, m * 128:(m + 1) * 128],
                                         y1[:, k, :],
                                         start=(k == 0), stop=(k == KC - 1))
                    nc.vector.tensor_tensor(
                        ob[:, m, :], ps[:],
                        bsb["b2"][:, m, None].to_broadcast([128, BLK]),
                        ALU.add)
                nc.sync.dma_start(OUT[:, :, tok0:tok0 + BLK], ob[:])

            for b in range(BPC):
                for sb in range(NSEN // SBLK):
                    tok0 = b * (NSEN * NSEG) + sb * BLK
                    xblk = blkp.tile([128, KC, BLK], bf16, tag="xblk", name="xblk")
                    for h in range(H):
                        combo = (b * H + h) * (NSEN // SBLK) + sb
                        q0 = 32 * (combo % 3)
                        off = (combo // 3) * BLK
                        pb = pbps.tile([128, BLK], fp32, tag="pb", name="pb")
                        nc.tensor.matmul(
                            pb[:], ones[q0:q0 + 1, :],
                            pflat[q0:q0 + 1, off:off + BLK],
                            start=True, stop=True)
                        x3 = xblk[:, h, :].rearrange("p (s g) -> p s g", s=SBLK)
                        dv = dVT[:, h, b * NSEG:(b + 1) * NSEG][:, None, :] \
                            .to_broadcast([128, SBLK, NSEG])
                        vv = vAVT[:, h, TOK_AV + b * NSEG:TOK_AV + (b + 1) * NSEG][:, None, :] \
                            .to_broadcast([128, SBLK, NSEG])
                        pb3 = pb[:].rearrange("p (s g) -> p s g", s=SBLK)
                        nc.vector.tensor_tensor(x3, pb3, dv, ALU.mult)
                        nc.vector.tensor_tensor(x3, x3, vv, ALU.add)
                    if pending is not None:
                        emit_ff(*pending)
                    pending = (xblk, tok0)
            emit_ff(*pending)

    nc.finalize()
    return nc


def _prep_core_inputs(inputs, core):
    b0 = core * BPC
    f32 = np.float32

    import ml_dtypes
    bf16 = ml_dtypes.bfloat16

    def t_act(x, ntok):
        # (bpc, n, D) -> [128, KC, ntok] with token = (b, n)
        flat = np.ascontiguousarray(x[b0:b0 + BPC]).reshape(ntok, KC, 128)
        return np.ascontiguousarray(flat.transpose(2, 1, 0)).astype(bf16)

    def t_w(w, dtype=f32):
        # (D, D) -> w.T as [128, KC, D]:  [p, k, n] = w[n, k*128+p]
        wt = np.ascontiguousarray(w.T).reshape(KC, 128, D)
        return np.ascontiguousarray(wt.transpose(1, 0, 2)).astype(dtype)

    def t_b(b):
        return np.ascontiguousarray(b.reshape(KC, 128).T, dtype=f32)

    m = {
        "AT": t_act(inputs["A"], TOK_AV),
        "VT": t_act(inputs["V"], TOK_AV),
        "ST": t_act(inputs["S"], TOK_S),
    }
    for w in ["wA", "wV", "wS", "wq", "wk", "wv", "w1", "w2"]:
        m[w] = t_w(inputs[w], bf16)
    for b in ["bA", "bV", "bS", "bq", "bk", "bv", "b1", "b2"]:
        m[b] = t_b(inputs[b])
    return m


def kernel(**inputs):
    import os
    from concourse.bass_utils import run_bass_kernel_spmd

    inputs = {k: np.asarray(v, dtype=np.float32) for k, v in inputs.items()}
    if "nc" not in _CACHE:
        _CACHE["nc"] = _build_nc()
    nc = _CACHE["nc"]

    in_maps = [_prep_core_inputs(inputs, c) for c in range(NCORES)]
    trace = os.environ.get("TRACE", "0") == "1"
    res = run_bass_kernel_spmd(nc, in_maps, core_ids=list(range(NCORES)),
                               trace=trace)
    _CACHE["last_results"] = res

    out = np.empty((BS, NSEN, NSEG, D), dtype=np.float32)
    for c in range(NCORES):
        oc = res.results[c]["OUT"]  # [128, KC, TOK_OUT]
        # element [p, m, (b, s, g)] = out[b0+b, s, g, m*128+p]
        oc = oc.reshape(128, KC, BPC, NSEN, NSEG).transpose(2, 3, 4, 1, 0)
        out[c * BPC:(c + 1) * BPC] = oc.reshape(BPC, NSEN, NSEG, D)
    return out

